# revision 9
# baseline (speedup 1.0000x reference)
"""Trainium2 Bass kernel for nn_Chf_Likelihood_Loss.

Reference computes, for B=8 density maps of H=W=64:
    loss = mean_b sum_ij |CHF_ij(out_b) - CHF_ij(gt_b)|^2
where CHF_ij(m) = sum_n exp(I*(f_j*x_n + f_i*y_n)) m_n over the N=4096 pixels
and (f_i) are 2S=60 frequencies.

Two algebraic reductions make this tiny:
  1. CHF is linear in the map, so CHF(out) - CHF(gt) = CHF(out - gt).
  2. The angle f_j*x_w + f_i*y_h is separable, so the [60,60,4096] template
     contraction factorizes into two skinny matmuls against [64,60] cos/sin
     factor matrices:
        A[i,w] = sum_h cos(f_i y_h) D[h,w],  Bm[i,w] = sum_h sin(f_i y_h) D[h,w]
        R = A@CxT - Bm@SxT,  I = A@SxT + Bm@CxT      (CxT[w,j] = cos(f_j x_w))
        loss_b = sum(R^2 + I^2)

Sharding: data-parallel over batch, one map per NeuronCore (B == 8 == n_cores).
Per core, all inputs (map pair + trig constants) arrive as ONE [64,308] DMA,
then 5 small matmuls + 4 DVE ops produce a [60,1] per-frequency-row partial;
the host sums partials and divides by B.
"""

import numpy as np

import concourse.bacc as bacc
import concourse.bass as bass
import concourse.tile as tile
from concourse import mybir
from concourse.bass_utils import run_bass_kernel_spmd

B, H, W = 8, 64, 64
CHF_STEP = 30
CHF_TIK = 0.01
SAMPLE_STEP = 1.0
SCALE = 1.0
S2 = 2 * CHF_STEP  # 60 frequencies
N_CORES = 8

# blob column layout: [ d | g | CT | ST | -ST ]
_C_D = 0
_C_G = W
_C_CT = 2 * W
_C_ST = 2 * W + S2
_C_NST = 2 * W + 2 * S2
_C_END = 2 * W + 3 * S2  # 308

_F32 = mybir.dt.float32


def _make_trig() -> np.ndarray:
    """[W, 3*S2] = [CT | ST | -ST] with CT[w, j] = cos(f_j * x_w).

    x_axis == y_axis here (H == W, same sampling), so the same matrix serves
    the stage-1 (y) and stage-2 (x) contractions.
    """
    half = SAMPLE_STEP / 2
    x_axis = np.linspace(half, W * SAMPLE_STEP - half, W).astype(np.float32)
    freqs = (np.arange(-CHF_STEP, CHF_STEP) * CHF_TIK).astype(np.float32)
    ang = np.outer(x_axis, freqs).astype(np.float32)  # [W, S2]
    ct = np.cos(ang).astype(np.float32)
    st = np.sin(ang).astype(np.float32)
    return np.concatenate([ct, st, -st], axis=1)  # [W, 3*S2]


def _build_bass() -> bass.Bass:
    nc = bacc.Bacc("TRN2", target_bir_lowering=False, debug=False, num_devices=N_CORES)

    blob_in = nc.dram_tensor("blob", [H, _C_END], _F32, kind="ExternalInput")
    o_out = nc.dram_tensor("o", [S2, 1], _F32, kind="ExternalOutput")

    with tile.TileContext(nc) as tc:
        with (
            tc.tile_pool(name="sb", bufs=1) as sb,
            tc.tile_pool(name="ps", bufs=1, space="PSUM") as ps,
        ):
            blob = sb.tile([H, _C_END], _F32)
            nc.sync.dma_start(blob[:], blob_in[:])

            # Stage 1 on BOTH maps at once (keeps the first PE op's only
            # dependency the input DMA — a Matmult can carry just one sync
            # wait): ps1 = [d|g].T @ [CT|ST] -> [2W, 2*S2]
            # rows 0:W = [A_d^T | B_d^T], rows W:2W = [A_g^T | B_g^T]
            ps1 = ps.tile([2 * W, 2 * S2], _F32)
            nc.tensor.matmul(
                ps1[:],
                blob[:, _C_D : _C_D + 2 * W],
                blob[:, _C_CT : _C_CT + 2 * S2],
                start=True,
                stop=True,
            )
            # Linearity: transform(out) - transform(gt) = transform(out - gt).
            # DVE may read at most one PSUM operand, so bounce the top half
            # to SBUF first, then subtract the bottom half (single PSUM read).
            u = sb.tile([W, 2 * S2], _F32)
            nc.vector.tensor_copy(u[:], ps1[0:W, :])
            s1 = sb.tile([W, 2 * S2], _F32)
            nc.vector.tensor_sub(s1[:], u[:], ps1[W : 2 * W, :])

            # Stage 2: [R | I] in one PSUM tile [S2, 2*S2]
            #   R = A@CT + Bm@(-ST),  I = A@ST + Bm@CT
            at = s1[:, 0:S2]
            bt = s1[:, S2 : 2 * S2]
            ct = blob[:, _C_CT : _C_CT + S2]
            st = blob[:, _C_ST : _C_ST + S2]
            nst = blob[:, _C_NST : _C_NST + S2]
            ps2 = ps.tile([S2, 2 * S2], _F32)
            nc.tensor.matmul(ps2[:, 0:S2], at, ct, start=True, stop=False)
            nc.tensor.matmul(ps2[:, 0:S2], bt, nst, start=False, stop=True)
            nc.tensor.matmul(ps2[:, S2 : 2 * S2], at, st, start=True, stop=False)
            nc.tensor.matmul(ps2[:, S2 : 2 * S2], bt, ct, start=False, stop=True)

            # Fused square + row-sum: acc[i] = sum_j ps2[i,j]^2
            # (DVE may read at most one PSUM operand, so bounce ps2 to SBUF.)
            c2 = sb.tile([S2, 2 * S2], _F32)
            nc.vector.tensor_copy(c2[:], ps2[:])
            sq = sb.tile([S2, 2 * S2], _F32)
            nc.vector.tensor_mul(sq[:], c2[:], c2[:])
            acc = sb.tile([S2, 1], _F32)
            nc.vector.reduce_sum(acc[:], sq[:], axis=mybir.AxisListType.X)

            nc.sync.dma_start(o_out[:], acc[:])

    nc.compile()
    return nc


def _run(inputs: dict, trace: bool = False):
    dnn = np.ascontiguousarray(np.asarray(inputs["dnn_output"], dtype=np.float32))
    gt = np.ascontiguousarray(np.asarray(inputs["gt_density_map"], dtype=np.float32))
    assert dnn.shape == (B, H, W) and gt.shape == (B, H, W)

    trig = _make_trig()
    nc = _build_bass()
    in_maps = [
        {"blob": np.concatenate([dnn[b], gt[b], trig], axis=1)} for b in range(B)
    ]
    res = run_bass_kernel_spmd(nc, in_maps, list(range(N_CORES)), trace=trace)
    partials = [res.results[b]["o"] for b in range(B)]
    total = np.sum(np.stack(partials, axis=0), dtype=np.float64)
    loss = np.float32(total / B * SCALE)
    return np.asarray(loss, dtype=np.float32), res


def kernel(**inputs) -> np.ndarray:
    loss, _ = _run(inputs, trace=False)
    return loss


# revision 10
# speedup vs baseline: 1.0286x; 1.0286x over previous
"""Trainium2 Bass kernel for nn_Chf_Likelihood_Loss.

Reference computes, for B=8 density maps of H=W=64:
    loss = mean_b sum_ij |CHF_ij(out_b) - CHF_ij(gt_b)|^2
where CHF_ij(m) = sum_n exp(I*(f_j*x_n + f_i*y_n)) m_n over the N=4096 pixels
and (f_i) are 2S=60 frequencies.

Two algebraic reductions make this tiny:
  1. CHF is linear in the map, so CHF(out) - CHF(gt) = CHF(out - gt).
  2. The angle f_j*x_w + f_i*y_h is separable, so the [60,60,4096] template
     contraction factorizes into skinny matmuls against [64,60] cos/sin
     factor matrices:
        A[i,w] = sum_h cos(f_i y_h) D[h,w],  Bm[i,w] = sum_h sin(f_i y_h) D[h,w]
        R = A@CxT - Bm@SxT,  I = A@SxT + Bm@CxT      (CxT[w,j] = cos(f_j x_w))
        loss_b = sum(R^2 + I^2)

Sharding: data-parallel over batch, one map per NeuronCore (B == 8 == n_cores).
Per core, everything (map pair + trig constants) arrives as ONE [64,368] DMA:
    blob = [ d | g | CT | ST | -ST | CT ]
Stage 1 runs on both maps at once (linearity lets us subtract the transforms
afterwards), stage 2 is two N=120 matmuls, and a DVE square+row-reduce yields
a [60,1] partial that the host sums across cores and divides by B.

Raw bacc (no TileContext): the Tile tail drain/EVSEM butterfly costs ~15us,
an order of magnitude more than this kernel's work, so semaphores are manual.
"""

import numpy as np

import concourse.bacc as bacc
import concourse.bass as bass
from concourse import mybir

B, H, W = 8, 64, 64
CHF_STEP = 30
CHF_TIK = 0.01
SAMPLE_STEP = 1.0
SCALE = 1.0
S2 = 2 * CHF_STEP  # 60 frequencies
N_CORES = 8

# blob column layout: [ d | g | CT | ST | -ST | CT ]
_C_D = 0
_C_G = W
_C_T1 = 2 * W            # [CT|ST], stage-1 rhs (N=120)
_C_T2 = 2 * W + 2 * S2   # [-ST|CT], stage-2 accumulate rhs (N=120)
_C_END = 2 * W + 4 * S2  # 368

_F32 = mybir.dt.float32


def _make_trig() -> np.ndarray:
    """[W, 4*S2] = [CT | ST | -ST | CT] with CT[w, j] = cos(f_j * x_w).

    x_axis == y_axis here (H == W, same sampling), so the same matrix serves
    the stage-1 (y) and stage-2 (x) contractions.
    """
    half = SAMPLE_STEP / 2
    x_axis = np.linspace(half, W * SAMPLE_STEP - half, W).astype(np.float32)
    freqs = (np.arange(-CHF_STEP, CHF_STEP) * CHF_TIK).astype(np.float32)
    ang = np.outer(x_axis, freqs).astype(np.float32)  # [W, S2]
    ct = np.cos(ang).astype(np.float32)
    st = np.sin(ang).astype(np.float32)
    return np.concatenate([ct, st, -st, ct], axis=1)  # [W, 4*S2]


def _build_bass() -> bass.Bass:
    nc = bacc.Bacc("TRN2", target_bir_lowering=False, debug=False, num_devices=N_CORES)

    blob_in = nc.dram_tensor("blob", [H, _C_END], _F32, kind="ExternalInput")
    o_out = nc.dram_tensor("o", [S2, 1], _F32, kind="ExternalOutput")

    with (
        nc.sbuf_tensor([H, _C_END], _F32) as blob,
        nc.sbuf_tensor([W, 2 * S2], _F32) as u,
        nc.sbuf_tensor([W, 2 * S2], _F32) as s1,
        nc.sbuf_tensor([S2, 2 * S2], _F32) as c2,
        nc.sbuf_tensor([S2, 2 * S2], _F32) as sq,
        nc.sbuf_tensor([S2, 1], _F32) as acc,
        nc.psum_tensor([2 * W, 2 * S2], _F32) as ps1,
        nc.psum_tensor([S2, 2 * S2], _F32) as ps2,
        nc.semaphore("dma_in") as dma_in_sem,
        nc.semaphore("pe") as pe_sem,
        nc.semaphore("dve") as dve_sem,
        nc.semaphore("dma_out") as dma_out_sem,
        nc.Block() as block,
    ):

        @block.sync
        def _(sync):
            sync.dma_start(out=blob[:], in_=blob_in[:]).then_inc(dma_in_sem, 16)
            sync.wait_ge(dve_sem, 5)
            sync.dma_start(out=o_out[:], in_=acc[:]).then_inc(dma_out_sem, 16)
            # Restore all semaphores to 0 so the loaded NEFF can re-execute.
            sync.wait_ge(dma_out_sem, 16)
            sync.sem_clear(dma_in_sem)
            sync.sem_clear(pe_sem)
            sync.sem_clear(dve_sem)
            sync.sem_clear(dma_out_sem)

        @block.tensor
        def _(tensor):
            # Stage 1 on both maps: ps1 = [d|g].T @ [CT|ST] -> [2W, 120]
            # rows 0:W = [A_d^T|B_d^T], rows W:2W = [A_g^T|B_g^T]
            tensor.wait_ge(dma_in_sem, 16)
            nc.tensor.matmul(
                ps1[:],
                blob[:, _C_D : _C_D + 2 * W],
                blob[:, _C_T1 : _C_T1 + 2 * S2],
                start=True,
                stop=True,
            ).then_inc(pe_sem, 1)
            # Stage 2: ps2 = [R | I] = A@[CT|ST] + Bm@[-ST|CT]
            tensor.wait_ge(dve_sem, 2)
            nc.tensor.matmul(
                ps2[:],
                s1[:, 0:S2],
                blob[:, _C_T1 : _C_T1 + 2 * S2],
                start=True,
                stop=False,
            )
            nc.tensor.matmul(
                ps2[:],
                s1[:, S2 : 2 * S2],
                blob[:, _C_T2 : _C_T2 + 2 * S2],
                start=False,
                stop=True,
            ).then_inc(pe_sem, 1)

        @block.vector
        def _(vector):
            # Linearity: transform(d) - transform(g) = transform(d - g).
            # DVE may read only one PSUM operand per op, so bounce the top
            # half to SBUF, then subtract the bottom half.
            vector.wait_ge(pe_sem, 1)
            nc.vector.tensor_copy(u[:], ps1[0:W, :]).then_inc(dve_sem, 1)
            vector.wait_ge(dve_sem, 1)
            nc.vector.tensor_sub(s1[:], u[:], ps1[W : 2 * W, :]).then_inc(dve_sem, 1)
            # Square + row-sum: acc[i] = sum_j R[i,j]^2 + I[i,j]^2
            vector.wait_ge(pe_sem, 2)
            nc.vector.tensor_copy(c2[:], ps2[:]).then_inc(dve_sem, 1)
            vector.wait_ge(dve_sem, 3)
            nc.vector.tensor_mul(sq[:], c2[:], c2[:]).then_inc(dve_sem, 1)
            vector.wait_ge(dve_sem, 4)
            nc.vector.reduce_sum(acc[:], sq[:], axis=mybir.AxisListType.X).then_inc(
                dve_sem, 1
            )

    nc.compile()
    return nc


def _run(inputs: dict, trace: bool = False):
    from concourse.bass_utils import run_bass_kernel_spmd

    dnn = np.ascontiguousarray(np.asarray(inputs["dnn_output"], dtype=np.float32))
    gt = np.ascontiguousarray(np.asarray(inputs["gt_density_map"], dtype=np.float32))
    assert dnn.shape == (B, H, W) and gt.shape == (B, H, W)

    trig = _make_trig()
    nc = _build_bass()
    in_maps = [
        {"blob": np.concatenate([dnn[b], gt[b], trig], axis=1)} for b in range(B)
    ]
    res = run_bass_kernel_spmd(nc, in_maps, list(range(N_CORES)), trace=trace)
    partials = [res.results[b]["o"] for b in range(B)]
    total = np.sum(np.stack(partials, axis=0), dtype=np.float64)
    loss = np.float32(total / B * SCALE)
    return np.asarray(loss, dtype=np.float32), res


def kernel(**inputs) -> np.ndarray:
    loss, _ = _run(inputs, trace=False)
    return loss


# revision 14
# speedup vs baseline: 1.3597x; 1.3219x over previous
"""Trainium2 Bass kernel for nn_Chf_Likelihood_Loss.

Reference computes, for B=8 density maps of H=W=64:
    loss = mean_b sum_ij |CHF_ij(out_b) - CHF_ij(gt_b)|^2
where CHF_ij(m) = sum_n exp(I*(f_j*x_n + f_i*y_n)) m_n over the N=4096 pixels
and (f_i) are 2S=60 frequencies.

Two algebraic reductions make this tiny:
  1. CHF is linear in the map, so CHF(out) - CHF(gt) = CHF(out - gt).
  2. The angle f_j*x_w + f_i*y_h is separable, so the [60,60,4096] template
     contraction factorizes into skinny matmuls against [64,60] cos/sin
     factor matrices:
        A[i,w] = sum_h cos(f_i y_h) D[h,w],  Bm[i,w] = sum_h sin(f_i y_h) D[h,w]
        R = A@CxT - Bm@SxT,  I = A@SxT + Bm@CxT      (CxT[w,j] = cos(f_j x_w))
        loss_b = sum(R^2 + I^2)

Sharding: data-parallel over batch, one map per NeuronCore (B == 8 == n_cores).
Per core, everything (map pair + trig constants) arrives as ONE [64,368] DMA:
    blob = [ d | g | CT | ST | -ST | CT ]
Stage 1 runs on both maps at once (linearity lets us subtract the transforms
afterwards), stage 2 is two N=120 matmuls, and a DVE square+row-reduce yields
a [60,1] partial that the host sums across cores and divides by B.

Raw bacc (no TileContext): the Tile tail drain/EVSEM butterfly costs ~15us,
an order of magnitude more than this kernel's work, so semaphores are manual.
"""

import numpy as np

import concourse.bacc as bacc
import concourse.bass as bass
from concourse import mybir

B, H, W = 8, 64, 64
CHF_STEP = 30
CHF_TIK = 0.01
SAMPLE_STEP = 1.0
SCALE = 1.0
S2 = 2 * CHF_STEP  # 60 frequencies
N_CORES = 8

# blob column layout: [ d | g | CT | ST | -ST | CT | ones ]
_C_D = 0
_C_G = W
_C_T1 = 2 * W            # [CT|ST], stage-1 rhs (N=120)
_C_T2 = 2 * W + 2 * S2   # [-ST|CT], stage-2 accumulate rhs (N=120)
_C_ONE = 2 * W + 4 * S2  # ones column (partition-sum lhsT)
_C_END = _C_ONE + 1      # 369

_F32 = mybir.dt.float32


def _make_trig() -> np.ndarray:
    """[W, 4*S2] = [CT | ST | -ST | CT] with CT[w, j] = cos(f_j * x_w).

    x_axis == y_axis here (H == W, same sampling), so the same matrix serves
    the stage-1 (y) and stage-2 (x) contractions.
    """
    half = SAMPLE_STEP / 2
    x_axis = np.linspace(half, W * SAMPLE_STEP - half, W).astype(np.float32)
    freqs = (np.arange(-CHF_STEP, CHF_STEP) * CHF_TIK).astype(np.float32)
    ang = np.outer(x_axis, freqs).astype(np.float32)  # [W, S2]
    ct = np.cos(ang).astype(np.float32)
    st = np.sin(ang).astype(np.float32)
    ones = np.ones((W, 1), dtype=np.float32)
    return np.concatenate([ct, st, -st, ct, ones], axis=1)  # [W, 4*S2+1]


def _build_bass() -> bass.Bass:
    nc = bacc.Bacc("TRN2", target_bir_lowering=False, debug=False, num_devices=N_CORES)

    blob_in = nc.dram_tensor("blob", [H, _C_END], _F32, kind="ExternalInput")
    o_out = nc.dram_tensor("o", [1, 1], _F32, kind="ExternalOutput")

    with (
        nc.sbuf_tensor([H, _C_END], _F32) as blob,
        nc.sbuf_tensor([W, 2 * S2], _F32) as u,
        nc.sbuf_tensor([W, 2 * S2], _F32) as s1,
        nc.sbuf_tensor([S2, 2 * S2], _F32) as c2,
        nc.sbuf_tensor([S2, 2 * S2], _F32) as sq,
        nc.sbuf_tensor([1, 1], _F32) as acc,
        nc.psum_tensor([2 * W, 2 * S2], _F32) as ps1,
        nc.psum_tensor([S2, 2 * S2], _F32) as ps2,
        nc.psum_tensor([1, 2 * S2], _F32) as ps3,
        nc.semaphore("dma_in") as dma_in_sem,
        nc.semaphore("pe") as pe_sem,
        nc.semaphore("dve") as dve_sem,
        nc.semaphore("dma_out") as dma_out_sem,
        nc.Block() as block,
    ):

        @block.sync
        def _(sync):
            sync.dma_start(out=blob[:], in_=blob_in[:]).then_inc(dma_in_sem, 16)
            sync.wait_ge(dve_sem, 5)
            # [1,1] output: one descriptor; a [60,1] partition-strided store
            # costs ~7us in per-descriptor HBM-write latency.
            sync.dma_start(out=o_out[:], in_=acc[:]).then_inc(dma_out_sem, 16)
            # Restore all semaphores to 0 so the loaded NEFF can re-execute.
            sync.wait_ge(dma_out_sem, 16)
            sync.sem_clear(dma_in_sem)
            sync.sem_clear(pe_sem)
            sync.sem_clear(dve_sem)
            sync.sem_clear(dma_out_sem)

        @block.tensor
        def _(tensor):
            # Stage 1 on both maps: ps1 = [d|g].T @ [CT|ST] -> [2W, 120]
            # rows 0:W = [A_d^T|B_d^T], rows W:2W = [A_g^T|B_g^T]
            tensor.wait_ge(dma_in_sem, 16)
            nc.tensor.matmul(
                ps1[:],
                blob[:, _C_D : _C_D + 2 * W],
                blob[:, _C_T1 : _C_T1 + 2 * S2],
                start=True,
                stop=True,
            ).then_inc(pe_sem, 1)
            # Stage 2: ps2 = [R | I] = A@[CT|ST] + Bm@[-ST|CT]
            tensor.wait_ge(dve_sem, 2)
            nc.tensor.matmul(
                ps2[:],
                s1[:, 0:S2],
                blob[:, _C_T1 : _C_T1 + 2 * S2],
                start=True,
                stop=False,
            )
            nc.tensor.matmul(
                ps2[:],
                s1[:, S2 : 2 * S2],
                blob[:, _C_T2 : _C_T2 + 2 * S2],
                start=False,
                stop=True,
            ).then_inc(pe_sem, 1)
            # Partition sum of the squares: ps3 = ones.T @ sq -> [1, 120]
            tensor.wait_ge(dve_sem, 4)
            nc.tensor.matmul(
                ps3[:],
                blob[0:S2, _C_ONE : _C_ONE + 1],
                sq[:],
                start=True,
                stop=True,
            ).then_inc(pe_sem, 1)

        @block.vector
        def _(vector):
            # Linearity: transform(d) - transform(g) = transform(d - g).
            # DVE may read only one PSUM operand per op, so bounce the top
            # half to SBUF, then subtract the bottom half.
            vector.wait_ge(pe_sem, 1)
            nc.vector.tensor_copy(u[:], ps1[0:W, :]).then_inc(dve_sem, 1)
            vector.wait_ge(dve_sem, 1)
            nc.vector.tensor_sub(s1[:], u[:], ps1[W : 2 * W, :]).then_inc(dve_sem, 1)
            # Square: sq = [R|I]^2 elementwise
            vector.wait_ge(pe_sem, 2)
            nc.vector.tensor_copy(c2[:], ps2[:]).then_inc(dve_sem, 1)
            vector.wait_ge(dve_sem, 3)
            nc.vector.tensor_mul(sq[:], c2[:], c2[:]).then_inc(dve_sem, 1)
            # Final free-axis reduce of the [1, 120] partition sums
            vector.wait_ge(pe_sem, 3)
            nc.vector.reduce_sum(acc[:], ps3[:], axis=mybir.AxisListType.X).then_inc(
                dve_sem, 1
            )

    nc.compile()
    return nc


def _run(inputs: dict, trace: bool = False):
    from concourse.bass_utils import run_bass_kernel_spmd

    dnn = np.ascontiguousarray(np.asarray(inputs["dnn_output"], dtype=np.float32))
    gt = np.ascontiguousarray(np.asarray(inputs["gt_density_map"], dtype=np.float32))
    assert dnn.shape == (B, H, W) and gt.shape == (B, H, W)

    trig = _make_trig()
    nc = _build_bass()
    in_maps = [
        {"blob": np.concatenate([dnn[b], gt[b], trig], axis=1)} for b in range(B)
    ]
    res = run_bass_kernel_spmd(nc, in_maps, list(range(N_CORES)), trace=trace)
    total = np.sum(
        np.stack([res.results[b]["o"] for b in range(B)]), dtype=np.float64
    )
    loss = np.float32(total / B * SCALE)
    return np.asarray(loss, dtype=np.float32), res


def kernel(**inputs) -> np.ndarray:
    loss, _ = _run(inputs, trace=False)
    return loss


# revision 15
# speedup vs baseline: 1.3983x; 1.0284x over previous
"""Trainium2 Bass kernel for nn_Chf_Likelihood_Loss.

Reference computes, for B=8 density maps of H=W=64:
    loss = mean_b sum_ij |CHF_ij(out_b) - CHF_ij(gt_b)|^2
where CHF_ij(m) = sum_n exp(I*(f_j*x_n + f_i*y_n)) m_n over the N=4096 pixels
and (f_i) are 2S=60 frequencies.

Algebraic reductions that make this tiny:
  1. CHF is linear in the map, so CHF(out) - CHF(gt) = CHF(out - gt).
  2. The angle f_j*x_w + f_i*y_h is separable, so the [60,60,4096] template
     contraction factorizes into skinny matmuls against [64,60] cos/sin
     factor matrices:
        A[i,w] = sum_h cos(f_i y_h) D[h,w],  Bm[i,w] = sum_h sin(f_i y_h) D[h,w]
        R = A@CxT - Bm@SxT,  I = A@SxT + Bm@CxT      (CxT[w,j] = cos(f_j x_w))
        loss_b = sum(R^2 + I^2)
  3. The subtraction in (1) is folded into the stage-1 matmul: stack d on
     partitions 0:64 and g on 64:128, with rhs = [trig; -trig], so the K=128
     contraction emits the transform of (d - g) directly.

Sharding: data-parallel over batch, one map per NeuronCore (B == 8 == n_cores).
Per core, everything arrives as ONE [128, 305] DMA:
    blob rows 0:64  = [ d | CT | ST | -ST | CT | ones ]
    blob rows 64:128= [ g |-CT |-ST |  *  |  *  |  *  ]   (* = unused)
then: MM1 (K=128) -> s1 copy -> MM2 x2 -> square -> ones-matmul partition sum
-> scalar reduce -> one-descriptor [1,1] DMA out. Host sums the 8 partials
and divides by B.

Raw bacc (no TileContext): the Tile tail drain/EVSEM butterfly costs ~15us,
an order of magnitude more than this kernel's work, so semaphores are manual.
"""

import numpy as np

import concourse.bacc as bacc
import concourse.bass as bass
from concourse import mybir

B, H, W = 8, 64, 64
CHF_STEP = 30
CHF_TIK = 0.01
SAMPLE_STEP = 1.0
SCALE = 1.0
S2 = 2 * CHF_STEP  # 60 frequencies
N_CORES = 8

# blob column layout: [ map | CT | ST | -ST | CT | ones ]
_C_M = 0
_C_T1 = W                # [CT|ST] (negated on bottom half), stage-1 rhs (K=128, N=120)
_C_T2 = W + 2 * S2       # [-ST|CT], stage-2 accumulate rhs (K=64, N=120)
_C_ONE = W + 4 * S2      # ones column (partition-sum lhsT, K=60)
_C_END = _C_ONE + 1      # 305

_F32 = mybir.dt.float32


def _make_blob_consts() -> tuple[np.ndarray, np.ndarray]:
    """(top, bottom) constant column blocks [64, 241] each.

    top    = [ CT | ST | -ST | CT | ones ]
    bottom = [-CT |-ST |  0  |  0 | 0    ]
    with CT[w, j] = cos(f_j * x_w). x_axis == y_axis here (H == W, same
    sampling), so the same matrix serves the stage-1 (y) and stage-2 (x)
    contractions.
    """
    half = SAMPLE_STEP / 2
    x_axis = np.linspace(half, W * SAMPLE_STEP - half, W).astype(np.float32)
    freqs = (np.arange(-CHF_STEP, CHF_STEP) * CHF_TIK).astype(np.float32)
    ang = np.outer(x_axis, freqs).astype(np.float32)  # [W, S2]
    ct = np.cos(ang).astype(np.float32)
    st = np.sin(ang).astype(np.float32)
    ones = np.ones((W, 1), dtype=np.float32)
    zeros = np.zeros((W, 2 * S2 + 1), dtype=np.float32)
    top = np.concatenate([ct, st, -st, ct, ones], axis=1)
    bottom = np.concatenate([-ct, -st, zeros], axis=1)
    return top, bottom


def _build_bass() -> bass.Bass:
    nc = bacc.Bacc("TRN2", target_bir_lowering=False, debug=False, num_devices=N_CORES)

    blob_in = nc.dram_tensor("blob", [2 * H, _C_END], _F32, kind="ExternalInput")
    o_out = nc.dram_tensor("o", [1, 1], _F32, kind="ExternalOutput")

    with (
        nc.sbuf_tensor([2 * H, _C_END], _F32) as blob,
        nc.sbuf_tensor([W, 2 * S2], _F32) as s1,
        nc.sbuf_tensor([S2, 2 * S2], _F32) as c2,
        nc.sbuf_tensor([S2, 2 * S2], _F32) as sq,
        nc.sbuf_tensor([1, 1], _F32) as acc,
        nc.psum_tensor([W, 2 * S2], _F32) as ps1,
        nc.psum_tensor([S2, 2 * S2], _F32) as ps2,
        nc.psum_tensor([1, 2 * S2], _F32) as ps3,
        nc.semaphore("dma_in") as dma_in_sem,
        nc.semaphore("pe") as pe_sem,
        nc.semaphore("dve") as dve_sem,
        nc.semaphore("dma_out") as dma_out_sem,
        nc.Block() as block,
    ):

        @block.sync
        def _(sync):
            sync.dma_start(out=blob[:], in_=blob_in[:]).then_inc(dma_in_sem, 16)
            sync.wait_ge(dve_sem, 4)
            # [1,1] output: one descriptor; a [60,1] partition-strided store
            # costs ~7us in per-descriptor HBM-write latency.
            sync.dma_start(out=o_out[:], in_=acc[:]).then_inc(dma_out_sem, 16)
            # Restore all semaphores to 0 so the loaded NEFF can re-execute.
            sync.wait_ge(dma_out_sem, 16)
            sync.sem_clear(dma_in_sem)
            sync.sem_clear(pe_sem)
            sync.sem_clear(dve_sem)
            sync.sem_clear(dma_out_sem)

        @block.tensor
        def _(tensor):
            # Stage 1 with the subtraction folded in:
            # ps1 = [d;g].T @ [trig;-trig] -> [W, 120] = [A^T | Bm^T] of (d-g)
            tensor.wait_ge(dma_in_sem, 16)
            nc.tensor.matmul(
                ps1[:],
                blob[:, _C_M : _C_M + W],
                blob[:, _C_T1 : _C_T1 + 2 * S2],
                start=True,
                stop=True,
            ).then_inc(pe_sem, 1)
            # Stage 2: ps2 = [R | I] = A@[CT|ST] + Bm@[-ST|CT]  (K=64: top rows)
            tensor.wait_ge(dve_sem, 1)
            nc.tensor.matmul(
                ps2[:],
                s1[:, 0:S2],
                blob[0:W, _C_T1 : _C_T1 + 2 * S2],
                start=True,
                stop=False,
            )
            nc.tensor.matmul(
                ps2[:],
                s1[:, S2 : 2 * S2],
                blob[0:W, _C_T2 : _C_T2 + 2 * S2],
                start=False,
                stop=True,
            ).then_inc(pe_sem, 1)
            # Partition sum of the squares: ps3 = ones.T @ sq -> [1, 120]
            tensor.wait_ge(dve_sem, 3)
            nc.tensor.matmul(
                ps3[:],
                blob[0:S2, _C_ONE : _C_ONE + 1],
                sq[:],
                start=True,
                stop=True,
            ).then_inc(pe_sem, 1)

        @block.vector
        def _(vector):
            # Matmul lhsT must live in SBUF, so bounce ps1 once.
            vector.wait_ge(pe_sem, 1)
            nc.vector.tensor_copy(s1[:], ps1[:]).then_inc(dve_sem, 1)
            # Square: sq = [R|I]^2 elementwise (one PSUM operand max per op)
            vector.wait_ge(pe_sem, 2)
            nc.vector.tensor_copy(c2[:], ps2[:]).then_inc(dve_sem, 1)
            vector.wait_ge(dve_sem, 2)
            nc.vector.tensor_mul(sq[:], c2[:], c2[:]).then_inc(dve_sem, 1)
            # Final free-axis reduce of the [1, 120] partition sums
            vector.wait_ge(pe_sem, 3)
            nc.vector.reduce_sum(acc[:], ps3[:], axis=mybir.AxisListType.X).then_inc(
                dve_sem, 1
            )

    nc.compile()
    return nc


def _run(inputs: dict, trace: bool = False):
    from concourse.bass_utils import run_bass_kernel_spmd

    dnn = np.ascontiguousarray(np.asarray(inputs["dnn_output"], dtype=np.float32))
    gt = np.ascontiguousarray(np.asarray(inputs["gt_density_map"], dtype=np.float32))
    assert dnn.shape == (B, H, W) and gt.shape == (B, H, W)

    top, bottom = _make_blob_consts()
    nc = _build_bass()
    in_maps = []
    for b in range(B):
        blob = np.concatenate(
            [
                np.concatenate([dnn[b], top], axis=1),
                np.concatenate([gt[b], bottom], axis=1),
            ],
            axis=0,
        )  # [128, 305]
        in_maps.append({"blob": np.ascontiguousarray(blob)})
    res = run_bass_kernel_spmd(nc, in_maps, list(range(N_CORES)), trace=trace)
    total = np.sum(
        np.stack([res.results[b]["o"] for b in range(B)]), dtype=np.float64
    )
    loss = np.float32(total / B * SCALE)
    return np.asarray(loss, dtype=np.float32), res


def kernel(**inputs) -> np.ndarray:
    loss, _ = _run(inputs, trace=False)
    return loss


# revision 16
# speedup vs baseline: 1.8754x; 1.3412x over previous
"""Trainium2 Bass kernel for nn_Chf_Likelihood_Loss.

Reference computes, for B=8 density maps of H=W=64:
    loss = mean_b sum_ij |CHF_ij(out_b) - CHF_ij(gt_b)|^2
where CHF_ij(m) = sum_n exp(I*(f_j*x_n + f_i*y_n)) m_n over the N=4096 pixels
and (f_i) are 2S=60 frequencies.

Algebraic reductions that make this tiny:
  1. CHF is linear in the map, so CHF(out) - CHF(gt) = CHF(out - gt).
  2. The angle f_j*x_w + f_i*y_h is separable, so the [60,60,4096] template
     contraction factorizes into skinny matmuls against [64,60] cos/sin
     factor matrices:
        A[i,w] = sum_h cos(f_i y_h) D[h,w],  Bm[i,w] = sum_h sin(f_i y_h) D[h,w]
        R = A@CxT - Bm@SxT,  I = A@SxT + Bm@CxT      (CxT[w,j] = cos(f_j x_w))
        loss_b = sum(R^2 + I^2)
  3. The subtraction in (1) is folded into the stage-1 matmul: stack d on
     partitions 0:64 and g on 64:128, with rhs = [trig; -trig], so the K=128
     contraction emits the transform of (d - g) directly.

Sharding: data-parallel over batch, one map per NeuronCore (B == 8 == n_cores).
Per core, everything arrives as ONE [128, 305] DMA:
    blob rows 0:64  = [ d | CT | ST | -ST | CT | ones ]
    blob rows 64:128= [ g |-CT |-ST |  *  |  *  |  *  ]   (* = unused)
then: MM1 (K=128) -> s1 copy -> MM2 x2 -> square -> ones-matmul partition sum
-> scalar reduce -> one-descriptor [1,1] DMA out. Host sums the 8 partials
and divides by B.

Raw bacc (no TileContext): the Tile tail drain/EVSEM butterfly costs ~15us,
an order of magnitude more than this kernel's work, so semaphores are manual.
"""

import numpy as np

import concourse.bacc as bacc
import concourse.bass as bass
from concourse import mybir

B, H, W = 8, 64, 64
CHF_STEP = 30
CHF_TIK = 0.01
SAMPLE_STEP = 1.0
SCALE = 1.0
S2 = 2 * CHF_STEP  # 60 frequencies
N_CORES = 8

# blob column layout: [ map | CT | ST | -ST | CT | ones ]
_C_M = 0
_C_T1 = W                # [CT|ST] (negated on bottom half), stage-1 rhs (K=128, N=120)
_C_T2 = W + 2 * S2       # [-ST|CT], stage-2 accumulate rhs (K=64, N=120)
_C_ONE = W + 4 * S2      # ones column (partition-sum lhsT, K=60)
_C_END = _C_ONE + 1      # 305

_F32 = mybir.dt.float32


def _make_blob_consts() -> tuple[np.ndarray, np.ndarray]:
    """(top, bottom) constant column blocks [64, 241] each.

    top    = [ CT | ST | -ST | CT | ones ]
    bottom = [-CT |-ST |  0  |  0 | 0    ]
    with CT[w, j] = cos(f_j * x_w). x_axis == y_axis here (H == W, same
    sampling), so the same matrix serves the stage-1 (y) and stage-2 (x)
    contractions.
    """
    half = SAMPLE_STEP / 2
    x_axis = np.linspace(half, W * SAMPLE_STEP - half, W).astype(np.float32)
    freqs = (np.arange(-CHF_STEP, CHF_STEP) * CHF_TIK).astype(np.float32)
    ang = np.outer(x_axis, freqs).astype(np.float32)  # [W, S2]
    ct = np.cos(ang).astype(np.float32)
    st = np.sin(ang).astype(np.float32)
    ones = np.ones((W, 1), dtype=np.float32)
    zeros = np.zeros((W, 2 * S2 + 1), dtype=np.float32)
    top = np.concatenate([ct, st, -st, ct, ones], axis=1)
    bottom = np.concatenate([-ct, -st, zeros], axis=1)
    return top, bottom


def _build_bass() -> bass.Bass:
    # Strip removable fixed overheads (~4.3us measured): the const-AP memsets
    # emitted in Bass.__init__ (this kernel never uses const APs) and the
    # bass-level all-engine barriers (init + Block exit). The data-dependency
    # semaphore chain below fully orders the kernel, and walrus's own NEFF
    # epilogue still drains + barriers every engine before the semaphore wipe.
    orig_barrier = bass.Bass.all_engine_barrier
    orig_memset = bass.BassGpSimd.memset
    bass.Bass.all_engine_barrier = lambda self, *a, **k: None
    bass.BassGpSimd.memset = lambda self, *a, **k: None
    try:
        nc = _build_bass_inner()
    finally:
        bass.Bass.all_engine_barrier = orig_barrier
        bass.BassGpSimd.memset = orig_memset
    return nc


def _build_bass_inner() -> bass.Bass:
    nc = bacc.Bacc("TRN2", target_bir_lowering=False, debug=False, num_devices=N_CORES)

    blob_in = nc.dram_tensor("blob", [2 * H, _C_END], _F32, kind="ExternalInput")
    o_out = nc.dram_tensor("o", [1, 1], _F32, kind="ExternalOutput")

    with (
        nc.sbuf_tensor([2 * H, _C_END], _F32) as blob,
        nc.sbuf_tensor([W, 2 * S2], _F32) as s1,
        nc.sbuf_tensor([S2, 2 * S2], _F32) as c2,
        nc.sbuf_tensor([S2, 2 * S2], _F32) as sq,
        nc.sbuf_tensor([1, 1], _F32) as acc,
        nc.psum_tensor([W, 2 * S2], _F32) as ps1,
        nc.psum_tensor([S2, 2 * S2], _F32) as ps2,
        nc.psum_tensor([1, 2 * S2], _F32) as ps3,
        nc.semaphore("dma_in") as dma_in_sem,
        nc.semaphore("pe") as pe_sem,
        nc.semaphore("dve") as dve_sem,
        nc.semaphore("dma_out") as dma_out_sem,
        nc.Block() as block,
    ):

        @block.sync
        def _(sync):
            sync.dma_start(out=blob[:], in_=blob_in[:]).then_inc(dma_in_sem, 16)
            sync.wait_ge(dve_sem, 4)
            # [1,1] output: one descriptor; a [60,1] partition-strided store
            # costs ~7us in per-descriptor HBM-write latency.
            sync.dma_start(out=o_out[:], in_=acc[:]).then_inc(dma_out_sem, 16)
            # Restore all semaphores to 0 so the loaded NEFF can re-execute.
            sync.wait_ge(dma_out_sem, 16)
            sync.sem_clear(dma_in_sem)
            sync.sem_clear(pe_sem)
            sync.sem_clear(dve_sem)
            sync.sem_clear(dma_out_sem)

        @block.tensor
        def _(tensor):
            # Stage 1 with the subtraction folded in:
            # ps1 = [d;g].T @ [trig;-trig] -> [W, 120] = [A^T | Bm^T] of (d-g)
            tensor.wait_ge(dma_in_sem, 16)
            nc.tensor.matmul(
                ps1[:],
                blob[:, _C_M : _C_M + W],
                blob[:, _C_T1 : _C_T1 + 2 * S2],
                start=True,
                stop=True,
            ).then_inc(pe_sem, 1)
            # Stage 2: ps2 = [R | I] = A@[CT|ST] + Bm@[-ST|CT]  (K=64: top rows)
            tensor.wait_ge(dve_sem, 1)
            nc.tensor.matmul(
                ps2[:],
                s1[:, 0:S2],
                blob[0:W, _C_T1 : _C_T1 + 2 * S2],
                start=True,
                stop=False,
            )
            nc.tensor.matmul(
                ps2[:],
                s1[:, S2 : 2 * S2],
                blob[0:W, _C_T2 : _C_T2 + 2 * S2],
                start=False,
                stop=True,
            ).then_inc(pe_sem, 1)
            # Partition sum of the squares: ps3 = ones.T @ sq -> [1, 120]
            tensor.wait_ge(dve_sem, 3)
            nc.tensor.matmul(
                ps3[:],
                blob[0:S2, _C_ONE : _C_ONE + 1],
                sq[:],
                start=True,
                stop=True,
            ).then_inc(pe_sem, 1)

        @block.vector
        def _(vector):
            # Matmul lhsT must live in SBUF, so bounce ps1 once.
            vector.wait_ge(pe_sem, 1)
            nc.vector.tensor_copy(s1[:], ps1[:]).then_inc(dve_sem, 1)
            # Square: sq = [R|I]^2 elementwise (one PSUM operand max per op)
            vector.wait_ge(pe_sem, 2)
            nc.vector.tensor_copy(c2[:], ps2[:]).then_inc(dve_sem, 1)
            vector.wait_ge(dve_sem, 2)
            nc.vector.tensor_mul(sq[:], c2[:], c2[:]).then_inc(dve_sem, 1)
            # Final free-axis reduce of the [1, 120] partition sums
            vector.wait_ge(pe_sem, 3)
            nc.vector.reduce_sum(acc[:], ps3[:], axis=mybir.AxisListType.X).then_inc(
                dve_sem, 1
            )

    nc.compile()
    return nc


def _run(inputs: dict, trace: bool = False):
    from concourse.bass_utils import run_bass_kernel_spmd

    dnn = np.ascontiguousarray(np.asarray(inputs["dnn_output"], dtype=np.float32))
    gt = np.ascontiguousarray(np.asarray(inputs["gt_density_map"], dtype=np.float32))
    assert dnn.shape == (B, H, W) and gt.shape == (B, H, W)

    top, bottom = _make_blob_consts()
    nc = _build_bass()
    in_maps = []
    for b in range(B):
        blob = np.concatenate(
            [
                np.concatenate([dnn[b], top], axis=1),
                np.concatenate([gt[b], bottom], axis=1),
            ],
            axis=0,
        )  # [128, 305]
        in_maps.append({"blob": np.ascontiguousarray(blob)})
    res = run_bass_kernel_spmd(nc, in_maps, list(range(N_CORES)), trace=trace)
    total = np.sum(
        np.stack([res.results[b]["o"] for b in range(B)]), dtype=np.float64
    )
    loss = np.float32(total / B * SCALE)
    return np.asarray(loss, dtype=np.float32), res


def kernel(**inputs) -> np.ndarray:
    loss, _ = _run(inputs, trace=False)
    return loss


# revision 23
# speedup vs baseline: 1.9075x; 1.0171x over previous
"""Trainium2 Bass kernel for nn_Chf_Likelihood_Loss.

Reference computes, for B=8 density maps of H=W=64:
    loss = mean_b sum_ij |CHF_ij(out_b) - CHF_ij(gt_b)|^2
where CHF_ij(m) = sum_n exp(I*(f_j*x_n + f_i*y_n)) m_n over the N=4096 pixels
and (f_i) are 2S=60 frequencies.

Algebraic reductions that make this tiny:
  1. CHF is linear in the map, so CHF(out) - CHF(gt) = CHF(out - gt).
  2. The angle f_j*x_w + f_i*y_h is separable, so the [60,60,4096] template
     contraction factorizes into skinny matmuls against [64,60] cos/sin
     factor matrices:
        A[i,w] = sum_h cos(f_i y_h) D[h,w],  Bm[i,w] = sum_h sin(f_i y_h) D[h,w]
        R = A@CxT - Bm@SxT,  I = A@SxT + Bm@CxT      (CxT[w,j] = cos(f_j x_w))
        loss_b = sum(R^2 + I^2)
  3. The subtraction in (1) is folded into the stage-1 matmul: stack d on
     partitions 0:64 and g on 64:128, with rhs = [trig; -trig], so the K=128
     contraction emits the transform of (d - g) directly.

Sharding: data-parallel over batch, one map per NeuronCore (B == 8 == n_cores).
Per core, everything arrives as ONE [128, 305] DMA:
    blob rows 0:64  = [ d | CT | ST | -ST | CT | ones ]
    blob rows 64:128= [ g |-CT |-ST |  *  |  *  |  *  ]   (* = unused)
then: MM1 (K=128) -> s1 copy -> MM2 x2 -> square -> ones-matmul partition sum
-> scalar reduce -> one-descriptor [1,1] DMA out. Host sums the 8 partials
and divides by B.

Raw bacc (no TileContext): the Tile tail drain/EVSEM butterfly costs ~15us,
an order of magnitude more than this kernel's work, so semaphores are manual.
"""

import numpy as np

import concourse.bacc as bacc
import concourse.bass as bass
from concourse import mybir

B, H, W = 8, 64, 64
CHF_STEP = 30
CHF_TIK = 0.01
SAMPLE_STEP = 1.0
SCALE = 1.0
S2 = 2 * CHF_STEP  # 60 frequencies
N_CORES = 8

# blob column layout: [ map | T1 | T2 | ones ]
_C_M = 0
_C_T1 = W                # rows 0:64 = [CT|ST], rows 64:128 = [-CT|-ST]; stage-1 rhs (K=128, N=120)
_C_T2 = W + 2 * S2       # rows 0:64 = [CT|ST], rows 64:128 = [-ST| CT]; stage-2 rhs (K=128, N=120)
_C_ONE = W + 4 * S2      # ones column (partition-sum lhsT, K=60)
_C_END = _C_ONE + 1      # 305

_F32 = mybir.dt.float32


def _make_blob_consts() -> tuple[np.ndarray, np.ndarray]:
    """(top, bottom) constant column blocks [64, 241] each.

    top    = [ CT | ST | CT | ST | ones ]
    bottom = [-CT |-ST |-ST | CT | ones ]
    with CT[w, j] = cos(f_j * x_w). x_axis == y_axis here (H == W, same
    sampling), so the same matrix serves the stage-1 (y) and stage-2 (x)
    contractions.
    """
    half = SAMPLE_STEP / 2
    x_axis = np.linspace(half, W * SAMPLE_STEP - half, W).astype(np.float32)
    freqs = (np.arange(-CHF_STEP, CHF_STEP) * CHF_TIK).astype(np.float32)
    ang = np.outer(x_axis, freqs).astype(np.float32)  # [W, S2]
    ct = np.cos(ang).astype(np.float32)
    st = np.sin(ang).astype(np.float32)
    ones = np.ones((W, 1), dtype=np.float32)
    top = np.concatenate([ct, st, ct, st, ones], axis=1)
    bottom = np.concatenate([-ct, -st, -st, ct, ones], axis=1)
    return top, bottom


def _build_bass() -> bass.Bass:
    # Strip removable fixed overheads (~4.3us measured): the const-AP memsets
    # emitted in Bass.__init__ (this kernel never uses const APs) and the
    # bass-level all-engine barriers (init + Block exit). The data-dependency
    # semaphore chain below fully orders the kernel, and walrus's own NEFF
    # epilogue still drains + barriers every engine before the semaphore wipe.
    orig_barrier = bass.Bass.all_engine_barrier
    orig_memset = bass.BassGpSimd.memset
    bass.Bass.all_engine_barrier = lambda self, *a, **k: None
    bass.BassGpSimd.memset = lambda self, *a, **k: None
    try:
        nc = _build_bass_inner()
    finally:
        bass.Bass.all_engine_barrier = orig_barrier
        bass.BassGpSimd.memset = orig_memset
    return nc


def _build_bass_inner() -> bass.Bass:
    nc = bacc.Bacc("TRN2", target_bir_lowering=False, debug=False, num_devices=N_CORES)

    blob_in = nc.dram_tensor("blob", [2 * H, _C_END], _F32, kind="ExternalInput")
    o_out = nc.dram_tensor("o", [1, 1], _F32, kind="ExternalOutput")

    with (
        nc.sbuf_tensor([2 * H, _C_END], _F32) as blob,
        nc.sbuf_tensor([2 * W, S2], _F32) as s1,
        nc.sbuf_tensor([S2, 2 * S2], _F32) as c2,
        nc.sbuf_tensor([S2, 2 * S2], _F32) as sq,
        nc.sbuf_tensor([1, 1], _F32) as acc,
        nc.psum_tensor([W, 2 * S2], _F32) as ps1,
        nc.psum_tensor([S2, 2 * S2], _F32) as ps2,
        nc.psum_tensor([1, 2 * S2], _F32) as ps3,
        nc.semaphore("dma_in") as dma_in_sem,
        nc.semaphore("pe") as pe_sem,
        nc.semaphore("dve") as dve_sem,
        nc.semaphore("dma_out") as dma_out_sem,
        nc.Block() as block,
    ):

        @block.sync
        def _(sync):
            sync.dma_start(out=blob[:], in_=blob_in[:]).then_inc(dma_in_sem, 16)
            sync.wait_ge(dve_sem, 5)
            # [1,1] output: one descriptor; a [60,1] partition-strided store
            # costs ~7us in per-descriptor HBM-write latency.
            sync.dma_start(out=o_out[:], in_=acc[:]).then_inc(dma_out_sem, 16)
            # Restore all semaphores to 0 so the loaded NEFF can re-execute.
            sync.wait_ge(dma_out_sem, 16)
            sync.sem_clear(dma_in_sem)
            sync.sem_clear(pe_sem)
            sync.sem_clear(dve_sem)
            sync.sem_clear(dma_out_sem)

        @block.tensor
        def _(tensor):
            # Stage 1 with the subtraction folded in:
            # ps1 = [d;g].T @ [trig;-trig] -> [W, 120] = [A^T | Bm^T] of (d-g)
            tensor.wait_ge(dma_in_sem, 16)
            nc.tensor.matmul(
                ps1[:],
                blob[:, _C_M : _C_M + W],
                blob[:, _C_T1 : _C_T1 + 2 * S2],
                start=True,
                stop=True,
            ).then_inc(pe_sem, 1)
            # Stage 2 as ONE K=128 matmul: lhsT = [A^T; Bm^T] stacked on
            # partitions, rhs = [[CT|ST]; [-ST|CT]], so
            # ps2 = [R | I] = A@[CT|ST] + Bm@[-ST|CT]
            tensor.wait_ge(dve_sem, 2)
            nc.tensor.matmul(
                ps2[:],
                s1[:],
                blob[:, _C_T2 : _C_T2 + 2 * S2],
                start=True,
                stop=True,
            ).then_inc(pe_sem, 1)
            # Partition sum of the squares: ps3 = ones.T @ sq -> [1, 120]
            tensor.wait_ge(dve_sem, 4)
            nc.tensor.matmul(
                ps3[:],
                blob[0:S2, _C_ONE : _C_ONE + 1],
                sq[:],
                start=True,
                stop=True,
            ).then_inc(pe_sem, 1)

        @block.vector
        def _(vector):
            # Matmul lhsT must live in SBUF; re-layout [A^T | Bm^T] [64,120]
            # into [A^T; Bm^T] [128,60] while bouncing ps1 out of PSUM.
            vector.wait_ge(pe_sem, 1)
            nc.vector.tensor_copy(s1[0:W, :], ps1[:, 0:S2]).then_inc(dve_sem, 1)
            nc.vector.tensor_copy(s1[W : 2 * W, :], ps1[:, S2 : 2 * S2]).then_inc(
                dve_sem, 1
            )
            # Square: sq = [R|I]^2 elementwise (one PSUM operand max per op)
            vector.wait_ge(pe_sem, 2)
            nc.vector.tensor_copy(c2[:], ps2[:]).then_inc(dve_sem, 1)
            vector.wait_ge(dve_sem, 3)
            nc.vector.tensor_mul(sq[:], c2[:], c2[:]).then_inc(dve_sem, 1)
            # Final free-axis reduce of the [1, 120] partition sums
            vector.wait_ge(pe_sem, 3)
            nc.vector.reduce_sum(acc[:], ps3[:], axis=mybir.AxisListType.X).then_inc(
                dve_sem, 1
            )

    nc.compile()
    return nc


def _run(inputs: dict, trace: bool = False):
    from concourse.bass_utils import run_bass_kernel_spmd

    dnn = np.ascontiguousarray(np.asarray(inputs["dnn_output"], dtype=np.float32))
    gt = np.ascontiguousarray(np.asarray(inputs["gt_density_map"], dtype=np.float32))
    assert dnn.shape == (B, H, W) and gt.shape == (B, H, W)

    top, bottom = _make_blob_consts()
    nc = _build_bass()
    in_maps = []
    for b in range(B):
        blob = np.concatenate(
            [
                np.concatenate([dnn[b], top], axis=1),
                np.concatenate([gt[b], bottom], axis=1),
            ],
            axis=0,
        )  # [128, 305]
        in_maps.append({"blob": np.ascontiguousarray(blob)})
    res = run_bass_kernel_spmd(nc, in_maps, list(range(N_CORES)), trace=trace)
    total = np.sum(
        np.stack([res.results[b]["o"] for b in range(B)]), dtype=np.float64
    )
    loss = np.float32(total / B * SCALE)
    return np.asarray(loss, dtype=np.float32), res


def kernel(**inputs) -> np.ndarray:
    loss, _ = _run(inputs, trace=False)
    return loss


# revision 27
# speedup vs baseline: 2.1211x; 1.1120x over previous
"""Trainium2 Bass kernel for nn_Chf_Likelihood_Loss.

Reference computes, for B=8 density maps of H=W=64:
    loss = mean_b sum_ij |CHF_ij(out_b) - CHF_ij(gt_b)|^2
where CHF_ij(m) = sum_n exp(I*(f_j*x_n + f_i*y_n)) m_n over the N=4096 pixels
and (f_i) are 2S=60 frequencies.

Algebraic reductions that make this tiny:
  1. CHF is linear in the map, so CHF(out) - CHF(gt) = CHF(out - gt).
  2. The angle f_j*x_w + f_i*y_h is separable, so the [60,60,4096] template
     contraction factorizes into skinny matmuls against [64,60] cos/sin
     factor matrices:
        A[i,w] = sum_h cos(f_i y_h) D[h,w],  Bm[i,w] = sum_h sin(f_i y_h) D[h,w]
        R = A@CxT - Bm@SxT,  I = A@SxT + Bm@CxT      (CxT[w,j] = cos(f_j x_w))
        loss_b = sum(R^2 + I^2)
  3. The subtraction in (1) is folded into the stage-1 matmul: stack d on
     partitions 0:64 and g on 64:128, with rhs = [trig; -trig], so the K=128
     contraction emits the transform of (d - g) directly.

Sharding: data-parallel over batch, one map per NeuronCore (B == 8 == n_cores).
Per core, everything arrives as ONE [128, 305] DMA:
    blob rows 0:64  = [ d | CT | ST | -ST | CT | ones ]
    blob rows 64:128= [ g |-CT |-ST |  *  |  *  |  *  ]   (* = unused)
then: MM1 (K=128) -> s1 copy -> MM2 x2 -> square -> ones-matmul partition sum
-> scalar reduce -> one-descriptor [1,1] DMA out. Host sums the 8 partials
and divides by B.

Raw bacc (no TileContext): the Tile tail drain/EVSEM butterfly costs ~15us,
an order of magnitude more than this kernel's work, so semaphores are manual.
"""

import numpy as np

import concourse.bacc as bacc
import concourse.bass as bass
from concourse import mybir

B, H, W = 8, 64, 64
CHF_STEP = 30
CHF_TIK = 0.01
SAMPLE_STEP = 1.0
SCALE = 1.0
S2 = 2 * CHF_STEP  # 60 frequencies
N_CORES = 8

# blob column layout: [ map | T1 | T2 | ones ]
_C_M = 0
_C_T1 = W                # rows 0:64 = [CT|ST], rows 64:128 = [-CT|-ST]; stage-1 rhs (K=128, N=120)
_C_T2 = W + 2 * S2       # rows 0:64 = [CT|ST], rows 64:128 = [-ST| CT]; stage-2 rhs (K=128, N=120)
_C_ONE = W + 4 * S2      # ones column (partition-sum lhsT, K=60)
_C_END = _C_ONE + 1      # 305

_F32 = mybir.dt.float32
# fp16 operands: single-pass PE matmuls (fp32 runs dual-pass LOW_HIGH), half
# the DMA bytes, fp32 PSUM accumulation. End-to-end rel err ~2e-5 (simulated).
_F16 = mybir.dt.float16


def _make_blob_consts() -> tuple[np.ndarray, np.ndarray]:
    """(top, bottom) constant column blocks [64, 241] each.

    top    = [ CT | ST | CT | ST | ones ]
    bottom = [-CT |-ST |-ST | CT | ones ]
    with CT[w, j] = cos(f_j * x_w). x_axis == y_axis here (H == W, same
    sampling), so the same matrix serves the stage-1 (y) and stage-2 (x)
    contractions.
    """
    half = SAMPLE_STEP / 2
    x_axis = np.linspace(half, W * SAMPLE_STEP - half, W).astype(np.float32)
    freqs = (np.arange(-CHF_STEP, CHF_STEP) * CHF_TIK).astype(np.float32)
    ang = np.outer(x_axis, freqs).astype(np.float32)  # [W, S2]
    ct = np.cos(ang).astype(np.float32)
    st = np.sin(ang).astype(np.float32)
    ones = np.ones((W, 1), dtype=np.float32)
    top = np.concatenate([ct, st, ct, st, ones], axis=1)
    bottom = np.concatenate([-ct, -st, -st, ct, ones], axis=1)
    return top.astype(np.float16), bottom.astype(np.float16)


def _build_bass() -> bass.Bass:
    # Strip removable fixed overheads (~4.3us measured): the const-AP memsets
    # emitted in Bass.__init__ (this kernel never uses const APs) and the
    # bass-level all-engine barriers (init + Block exit). The data-dependency
    # semaphore chain below fully orders the kernel, and walrus's own NEFF
    # epilogue still drains + barriers every engine before the semaphore wipe.
    orig_barrier = bass.Bass.all_engine_barrier
    orig_memset = bass.BassGpSimd.memset
    bass.Bass.all_engine_barrier = lambda self, *a, **k: None
    bass.BassGpSimd.memset = lambda self, *a, **k: None
    try:
        nc = _build_bass_inner()
    finally:
        bass.Bass.all_engine_barrier = orig_barrier
        bass.BassGpSimd.memset = orig_memset
    return nc


def _build_bass_inner() -> bass.Bass:
    nc = bacc.Bacc("TRN2", target_bir_lowering=False, debug=False, num_devices=N_CORES)

    blob_in = nc.dram_tensor("blob", [2 * H, _C_END], _F16, kind="ExternalInput")
    o_out = nc.dram_tensor("o", [1, 1], _F32, kind="ExternalOutput")

    with (
        nc.sbuf_tensor([2 * H, _C_END], _F16) as blob,
        nc.sbuf_tensor([2 * W, S2], _F16) as s1,
        nc.sbuf_tensor([S2, 2 * S2], _F16) as c2,
        nc.sbuf_tensor([S2, 2 * S2], _F16) as sq,
        nc.sbuf_tensor([1, 1], _F32) as acc,
        nc.psum_tensor([W, 2 * S2], _F32) as ps1,
        nc.psum_tensor([S2, 2 * S2], _F32) as ps2,
        nc.psum_tensor([1, 2 * S2], _F32) as ps3,
        nc.semaphore("dma_in") as dma_in_sem,
        nc.semaphore("pe") as pe_sem,
        nc.semaphore("dve") as dve_sem,
        nc.semaphore("dma_out") as dma_out_sem,
        nc.Block() as block,
    ):

        @block.sync
        def _(sync):
            sync.dma_start(out=blob[:], in_=blob_in[:]).then_inc(dma_in_sem, 16)
            sync.wait_ge(dve_sem, 5)
            # [1,1] output: one descriptor; a [60,1] partition-strided store
            # costs ~7us in per-descriptor HBM-write latency.
            sync.dma_start(out=o_out[:], in_=acc[:]).then_inc(dma_out_sem, 16)
            # Hold the queue open until the output lands in HBM (NRT reads the
            # buffer as soon as all queues retire). Semaphores need no manual
            # reset: walrus's NEFF epilogue wipes the whole sem file for
            # re-execution.
            sync.wait_ge(dma_out_sem, 16)

        @block.tensor
        def _(tensor):
            # Stage 1 with the subtraction folded in:
            # ps1 = [d;g].T @ [trig;-trig] -> [W, 120] = [A^T | Bm^T] of (d-g)
            tensor.wait_ge(dma_in_sem, 16)
            nc.tensor.matmul(
                ps1[:],
                blob[:, _C_M : _C_M + W],
                blob[:, _C_T1 : _C_T1 + 2 * S2],
                start=True,
                stop=True,
            ).then_inc(pe_sem, 1)
            # Stage 2 as ONE K=128 matmul: lhsT = [A^T; Bm^T] stacked on
            # partitions, rhs = [[CT|ST]; [-ST|CT]], so
            # ps2 = [R | I] = A@[CT|ST] + Bm@[-ST|CT]
            tensor.wait_ge(dve_sem, 2)
            nc.tensor.matmul(
                ps2[:],
                s1[:],
                blob[:, _C_T2 : _C_T2 + 2 * S2],
                start=True,
                stop=True,
            ).then_inc(pe_sem, 1)
            # Partition sum of the squares: ps3 = ones.T @ sq -> [1, 120]
            tensor.wait_ge(dve_sem, 4)
            nc.tensor.matmul(
                ps3[:],
                blob[0:S2, _C_ONE : _C_ONE + 1],
                sq[:],
                start=True,
                stop=True,
            ).then_inc(pe_sem, 1)

        @block.vector
        def _(vector):
            # Matmul lhsT must live in SBUF; re-layout [A^T | Bm^T] [64,120]
            # into [A^T; Bm^T] [128,60] while bouncing ps1 out of PSUM.
            vector.wait_ge(pe_sem, 1)
            nc.vector.tensor_copy(s1[0:W, :], ps1[:, 0:S2]).then_inc(dve_sem, 1)
            nc.vector.tensor_copy(s1[W : 2 * W, :], ps1[:, S2 : 2 * S2]).then_inc(
                dve_sem, 1
            )
            # Square: sq = [R|I]^2 elementwise (one PSUM operand max per op)
            vector.wait_ge(pe_sem, 2)
            nc.vector.tensor_copy(c2[:], ps2[:]).then_inc(dve_sem, 1)
            vector.wait_ge(dve_sem, 3)
            nc.vector.tensor_mul(sq[:], c2[:], c2[:]).then_inc(dve_sem, 1)
            # Final free-axis reduce of the [1, 120] partition sums
            vector.wait_ge(pe_sem, 3)
            nc.vector.reduce_sum(acc[:], ps3[:], axis=mybir.AxisListType.X).then_inc(
                dve_sem, 1
            )

    nc.compile()
    return nc


def _run(inputs: dict, trace: bool = False):
    from concourse.bass_utils import run_bass_kernel_spmd

    dnn = np.ascontiguousarray(np.asarray(inputs["dnn_output"], dtype=np.float32))
    gt = np.ascontiguousarray(np.asarray(inputs["gt_density_map"], dtype=np.float32))
    assert dnn.shape == (B, H, W) and gt.shape == (B, H, W)

    top, bottom = _make_blob_consts()
    nc = _build_bass()
    in_maps = []
    for b in range(B):
        blob = np.concatenate(
            [
                np.concatenate([dnn[b].astype(np.float16), top], axis=1),
                np.concatenate([gt[b].astype(np.float16), bottom], axis=1),
            ],
            axis=0,
        )  # [128, 305] fp16
        in_maps.append({"blob": np.ascontiguousarray(blob)})
    res = run_bass_kernel_spmd(nc, in_maps, list(range(N_CORES)), trace=trace)
    total = np.sum(
        np.stack([res.results[b]["o"] for b in range(B)]), dtype=np.float64
    )
    loss = np.float32(total / B * SCALE)
    return np.asarray(loss, dtype=np.float32), res


def kernel(**inputs) -> np.ndarray:
    loss, _ = _run(inputs, trace=False)
    return loss
